# revision 25
# baseline (speedup 1.0000x reference)
"""GPT-J joint attention (B=1, S=2048, D=2048, H=16, HD=128) on 8 Trainium2
NeuronCores, tensor-parallel over heads (2 heads per core).

Per-core program (matmuls bf16 inputs, fp32 PSUM accumulation). Key
differences vs the first working version (221us):
  - Q/K projections + scores/exp/softmax bookkeeping run at HEAD-PAIR
    granularity: one [128, 2, 512] PSUM tile (2 banks) holds both heads, so
    ACT/DVE ops amortize their fixed overhead across 2x the columns.
  - Causal narrowing: score/PV matmuls, exp and the causal-band mask skip
    the fully-masked column range of diagonal k-tiles.
  - Softmax denominator: per-kt partial sums in 2 bf16 lanes (DVE), merged,
    then ONE ones[128,128]-matmul per head yields the denominator already
    broadcast across all 128 partitions; reciprocal_approx_fast (single
    custom-DVE op, ~5x faster than InstReciprocal) replaces the 4us
    InstReciprocal + cast + PE-broadcast + copy chain.
  - All DMA'd tensors are pre-arranged host-side so every DMA descriptor is
    a large contiguous per-partition line (startup was descriptor-bound).
  - Out-proj copies split ACT/DVE; emission order keeps the DVE FIFO clear
    of the softmax-denominator critical path.

Host side: shard/transpose/cast inputs, run SPMD on 8 cores, sum the 8
partial outputs (the tensor-parallel all-reduce equivalent).
"""
import sys

import numpy as np
import ml_dtypes

try:
    import concourse.bass as bass
except ImportError:  # pragma: no cover
    sys.path.insert(0, "/opt/trn_rl_repo")
    import concourse.bass as bass

import concourse.mybir as mybir
import concourse.tile as tile
from concourse.bass_utils import run_bass_kernel_spmd

BF16 = mybir.dt.bfloat16
F32 = mybir.dt.float32
NPBF16 = ml_dtypes.bfloat16

N_CORES = 8
S = 2048          # sequence length
D = 2048          # model dim
HD = 128          # head dim
NHC = 2           # heads per core
DC = NHC * HD     # shard width (256)
P = 128           # partitions
KD = D // P       # 16 contraction tiles over model dim
QBS = 512         # q-block size
NQB = S // QBS    # 4 q-blocks
NST = S // P      # 16 sequence tiles of 128
SCALE = 1.0 / float(np.sqrt(HD))
OT_LAG = 2        # PV matmul trails the score matmul by this many kt pairs

# ---------------------------------------------------------------------------
# Walrus's CoreV3 drain encoding accepts a single sem wait; Tile's tail drain
# carries one wait per logical proc. Split it into one drain per proc.
# ---------------------------------------------------------------------------


def _install_drain_split():
    if getattr(tile.TileContext, "_drain_split_installed", False):
        return
    from concourse.vector_clock import ScopedClock, VectorClock

    def _drain_and_barrier(self, tick_clock, wait_clock):
        full = tick_clock.global_clock
        n = len(full)
        for i in range(n):
            if full[i] <= 0:
                continue
            vec = [full[j] if j == i else 0 for j in range(n)]
            drain_inst = self.nc.sync.drain()
            wait_clock.add_sem_waits(
                drain_inst.ins, ScopedClock({None: VectorClock(vec)})
            )
        self.nc.all_engine_barrier()
        assert self.sems is not None
        popped = self.nc._tile_sem_poison_stack.pop()
        assert popped is self._sem_poison
        self.nc.clear_and_free_semaphores(list(self.sems.allocated().values()))
        self.nc.all_engine_barrier()

    tile.TileContext._drain_and_barrier = _drain_and_barrier
    tile.TileContext._drain_split_installed = True


def _split_excess_waits(nc, limit=1):
    """This walrus build rejects instructions carrying more than one sem wait
    (CoreV3 setupSyncWait: 'Too many sync wait commands'). Spill excess waits
    onto same-engine NOPs inserted just before the instruction — the engine
    executes them in queue order, so blocking semantics are unchanged."""
    ctr = 0
    for fn in nc.m.functions:
        for blk in fn.blocks:
            new_list = []
            for inst in blk.instructions:
                si = inst.sync_info
                if si is not None and len(si.on_wait) > limit:
                    waits = list(si.on_wait)
                    excess, keep = waits[:-limit], waits[-limit:]
                    for w in excess:
                        ctr += 1
                        nop = mybir.InstNoOp(
                            name=f"I-wsplit-{ctr}", text_hint="wait_split"
                        )
                        nop.engine = inst.engine
                        nop.sync_info = mybir.SyncInfo(on_wait=[w], on_update=[])
                        new_list.append(nop)
                    inst.sync_info = mybir.SyncInfo(
                        on_wait=keep, on_update=si.on_update
                    )
                new_list.append(inst)
            if len(new_list) != len(blk.instructions):
                blk.instructions[:] = new_list
    return ctr


def build_nc(split_waits=True):
    _install_drain_split()
    nc = bass.Bass()

    # All DRAM tensors are pre-arranged host-side so each per-partition line
    # is contiguous (large DMA descriptors; startup was descriptor-bound).
    hT = nc.dram_tensor("hT", [P, NQB, KD, QBS], BF16, kind="ExternalInput")
    wq = nc.dram_tensor("wq", [P, KD, DC], BF16, kind="ExternalInput")
    wk = nc.dram_tensor("wk", [P, KD, DC], BF16, kind="ExternalInput")
    wv = nc.dram_tensor("wv", [P, KD, DC], BF16, kind="ExternalInput")
    wo = nc.dram_tensor("wo", [P, NHC, D], BF16, kind="ExternalInput")
    ct = nc.dram_tensor("ct", [P, S], BF16, kind="ExternalInput")
    st = nc.dram_tensor("st", [P, S], BF16, kind="ExternalInput")
    rot = nc.dram_tensor("rot", [P, P], BF16, kind="ExternalInput")
    out = nc.dram_tensor("out", [S, D], BF16, kind="ExternalOutput")

    Exp = mybir.ActivationFunctionType.Exp
    Copy = mybir.ActivationFunctionType.Copy

    with tile.TileContext(nc) as tc:
        with (
            tc.tile_pool(name="const", bufs=1) as const,
            tc.tile_pool(name="acts", bufs=1) as acts,
            tc.tile_pool(name="work", bufs=2) as work,
            tc.tile_pool(name="ptpool", bufs=8) as ptpool,
            tc.tile_pool(name="outstage", bufs=3) as outstage,
            tc.tile_pool(name="ps_pair", bufs=2, space="PSUM") as ps_pair,
            tc.tile_pool(name="ps_b", bufs=1, space="PSUM") as ps_b,
            tc.tile_pool(name="ps_c", bufs=2, space="PSUM") as ps_c,
        ):
            # ---- weights/constants into SBUF. Chunked so the very first
            # projection matmuls can start ~1.5us in; each chunk is a few
            # large contiguous per-partition descriptors. ----
            wq_sb = const.tile([P, KD, DC], BF16)
            wk_sb = const.tile([P, KD, DC], BF16)
            wv_sb = const.tile([P, KD, DC], BF16)
            hT_sb = const.tile([P, NQB, KD, QBS], BF16)
            ct_sb = const.tile([P, S], BF16)
            st_sb = const.tile([P, S], BF16)
            rot_sb = const.tile([P, P], BF16)
            wo_sb = const.tile([P, NHC, D], BF16)
            for c4 in range(4):
                ksl = slice(c4 * 4, (c4 + 1) * 4)
                nc.sync.dma_start(out=wq_sb[:, ksl, :], in_=wq[:, ksl, :])
                nc.sync.dma_start(
                    out=hT_sb[:, 0, ksl, :], in_=hT[:, 0, ksl, :]
                )
            nc.sync.dma_start(out=wk_sb, in_=wk[:, :, :])
            nc.sync.dma_start(out=rot_sb, in_=rot[:, :])
            nc.sync.dma_start(out=ct_sb, in_=ct[:, :])
            nc.sync.dma_start(out=st_sb, in_=st[:, :])
            nc.sync.dma_start(out=wv_sb, in_=wv[:, :, :])
            for qb in range(1, NQB):
                nc.sync.dma_start(
                    out=hT_sb[:, qb, :, :], in_=hT[:, qb, :, :]
                )
            nc.sync.dma_start(out=wo_sb, in_=wo[:, :, :])
            ones_sb = const.tile([P, P], BF16)  # lhsT for k-partition sums
            nc.vector.memset(ones_sb, 1.0)

            # persistent activations
            qt_sb = acts.tile([P, NHC, S], BF16)   # rotary-applied Q^T
            kt_sb = acts.tile([P, NHC, S], BF16)
            v_sb = acts.tile([P, NST, DC], BF16)   # [s%128, s//128, head*hd]
            otb_sb = acts.tile([P, NHC, S], BF16)  # normalized O^T per head

            def bcast(src, sl):
                return src[:, sl].unsqueeze(1).broadcast_to([P, NHC, QBS])

            # ---- projections, q-block at a time. Both heads of a q-block
            # share one [128, 2, 512] PSUM pair tile. The rope epilogue of
            # each pair is deferred until after the NEXT pair's matmuls are
            # emitted, so the PE never waits on it. ----
            pending = []  # (pair_psum, dst_sb, qb)

            def flush_rope(keep=0):
                while len(pending) > keep:
                    ps, dst_sb, qb = pending.pop(0)
                    sl = slice(qb * QBS, (qb + 1) * QBS)
                    raw = work.tile([P, NHC, QBS], BF16, tag="raw")
                    nc.scalar.activation(raw, ps, Copy)
                    rps = ps_b.tile([P, NHC, QBS], F32, tag="b", name="rps")
                    for h in range(NHC):
                        nc.tensor.matmul(
                            rps[:, h, :], lhsT=rot_sb, rhs=raw[:, h, :],
                            start=True, stop=True, skip_group_check=True,
                        )
                    t1 = work.tile([P, NHC, QBS], BF16, tag="t1")
                    t2 = work.tile([P, NHC, QBS], BF16, tag="t2")
                    nc.vector.tensor_mul(t1, raw, bcast(ct_sb, sl))
                    nc.vector.tensor_mul(t2, rps, bcast(st_sb, sl))
                    nc.vector.tensor_add(dst_sb[:, 0:NHC, sl], t1, t2)

            def project(w_sb, dst_sb, qb):
                flush_rope(keep=1)
                ps = ps_pair.tile(
                    [P, NHC, QBS], F32, tag="pair", name="proj_ps"
                )
                for kd in range(KD):
                    for h in range(NHC):
                        nc.tensor.matmul(
                            ps[:, h, :],
                            lhsT=w_sb[:, kd, h * HD:(h + 1) * HD],
                            rhs=hT_sb[:, qb, kd, :],
                            start=(kd == 0),
                            stop=(kd == KD - 1),
                            skip_group_check=True,
                        )
                pending.append((ps, dst_sb, qb))

            # ---- attention emission state (interleaved with projections:
            # the first PREROLL score pairs of each q-block are emitted
            # early so the PE never waits on the exp->PV latency at block
            # starts) ----
            PREROLL = 2

            def make_attn_state(qb):
                kt_order = list(range(0, 4 * qb)) + [
                    4 * qb + 3, 4 * qb + 2, 4 * qb + 1, 4 * qb
                ]
                offs = {}
                bands = {}
                for idx, kt in enumerate(kt_order):
                    j = kt - 4 * qb
                    if j < 0:
                        offs[kt] = 0
                        bands[kt] = None
                    else:
                        # wrap tiles first => lane inits (idx 0,1) must be
                        # full width; diag descending => last tile is full.
                        off = 0 if idx < 2 else j * P
                        offs[kt] = off
                        bands[kt] = (off, (j + 1) * P)
                return {
                    "qb": qb, "kt_order": kt_order, "offs": offs,
                    "bands": bands, "lanes": [None, None], "pts": {},
                    "ot_ps": None, "emitted": 0, "pv_done": 0,
                }

            def emit_score(stt):
                qb = stt["qb"]
                idx = stt["emitted"]
                kt = stt["kt_order"][idx]
                off = stt["offs"][kt]
                sps = ps_pair.tile([P, NHC, QBS], F32, tag="pair", name="sps")
                for h in range(NHC):
                    nc.tensor.matmul(
                        sps[:, h, off:QBS],
                        lhsT=kt_sb[:, h, kt * P:(kt + 1) * P],
                        rhs=qt_sb[:, h, qb * QBS + off:(qb + 1) * QBS],
                        start=True,
                        stop=True,
                        skip_group_check=True,
                    )
                pt = ptpool.tile([P, NHC, QBS], BF16, tag="pt")
                nc.scalar.activation(
                    pt[:, :, off:QBS], sps[:, :, off:QBS], Exp, scale=SCALE
                )
                if stt["bands"][kt] is not None:
                    blo, bhi = stt["bands"][kt]
                    nc.gpsimd.affine_select(
                        out=pt[:, :, blo:bhi],
                        in_=pt[:, :, blo:bhi],
                        compare_op=mybir.AluOpType.is_ge,
                        fill=0.0,
                        base=qb * QBS - kt * P + blo,
                        pattern=[[0, NHC], [1, bhi - blo]],
                        channel_multiplier=-1,
                    )
                stt["pts"][kt] = pt
                li = idx % 2
                if stt["lanes"][li] is None:
                    lane = work.tile(
                        [P, NHC, QBS], BF16,
                        name=f"za{qb}_{li}", tag=f"za{li}",
                    )
                    nc.vector.tensor_copy(lane, pt)
                    stt["lanes"][li] = lane
                else:
                    nc.vector.tensor_add(
                        stt["lanes"][li][:, :, off:QBS],
                        stt["lanes"][li][:, :, off:QBS],
                        pt[:, :, off:QBS],
                    )
                stt["emitted"] = idx + 1

            def emit_pv(stt):
                qb = stt["qb"]
                idx = stt["pv_done"]
                kt = stt["kt_order"][idx]
                off = stt["offs"][kt]
                if stt["ot_ps"] is None:
                    stt["ot_ps"] = ps_b.tile(
                        [P, NHC, QBS], F32, tag="b", name="ot_ps"
                    )
                for h in range(NHC):
                    nc.tensor.matmul(
                        stt["ot_ps"][:, h, off:QBS],
                        lhsT=v_sb[:, kt, h * HD:(h + 1) * HD],
                        rhs=stt["pts"][kt][:, h, off:QBS],
                        start=(idx == 0),
                        stop=(idx == len(stt["kt_order"]) - 1),
                        skip_group_check=True,
                    )
                stt["pv_done"] = idx + 1

            attn = [make_attn_state(qb) for qb in range(NQB)]

            for qb in range(NQB):
                project(wq_sb, qt_sb, qb)
                project(wk_sb, kt_sb, qb)
                for s4 in range(4):
                    flush_rope(keep=1)
                    st_idx = qb * 4 + s4
                    ps = ps_c.tile([P, QBS], F32, tag="c", name="v_ps")
                    for kd in range(KD):
                        nc.tensor.matmul(
                            ps[:, 0:DC],
                            lhsT=hT_sb[:, qb, kd, s4 * P:(s4 + 1) * P],
                            rhs=wv_sb[:, kd, :],
                            start=(kd == 0),
                            stop=(kd == KD - 1),
                        )
                    nc.scalar.activation(v_sb[:, st_idx, :], ps[:, 0:DC], Copy)
                    # pre-roll attention(qb=0) score pairs between the last
                    # V-projection groups (rope of Q3/K3 was just flushed,
                    # so the pair-psum slots cycle cleanly).
                    if qb == NQB - 1 and s4 < PREROLL:
                        emit_score(attn[0])
            flush_rope()

            # ---- attention + out-projection ----
            def out_proj_st(qb, s4s, tail=False):
                # Copy schedule per st (attention windows): eb0/eb2->DVE,
                # eb1->ACT, eb3 split — ACT carries the exps there. In the
                # TAIL (tail=True) both engines are idle: split every copy
                # into parallel ACT+DVE halves so psum slots free ~2x faster
                # and the matmul groups never wait.
                for s4 in s4s:
                    st_idx = qb * 4 + s4
                    rsl = slice(st_idx * P, (st_idx + 1) * P)
                    ost = outstage.tile([P, D], BF16, tag="ost")
                    for eb in range(NQB):
                        ops = ps_c.tile([P, QBS], F32, tag="c", name="ops")
                        for h in range(NHC):
                            nc.tensor.matmul(
                                ops,
                                lhsT=otb_sb[:, h, st_idx * P:(st_idx + 1) * P],
                                rhs=wo_sb[:, h, eb * QBS:(eb + 1) * QBS],
                                start=(h == 0),
                                stop=(h == NHC - 1),
                            )
                        osl = ost[:, eb * QBS:(eb + 1) * QBS]
                        if tail or eb == 3:
                            hw = QBS // 2
                            nc.scalar.activation(
                                osl[:, 0:hw], ops[:, 0:hw], Copy
                            )
                            nc.vector.tensor_copy(
                                osl[:, hw:QBS], ops[:, hw:QBS]
                            )
                        elif eb == 0 or eb == 2:
                            nc.vector.tensor_copy(osl, ops)
                        else:
                            nc.scalar.activation(osl, ops, Copy)
                        if eb == 1:
                            nc.sync.dma_start(
                                out=out[rsl, 0:2 * QBS], in_=ost[:, 0:2 * QBS]
                            )
                    nc.sync.dma_start(
                        out=out[rsl, 2 * QBS:D], in_=ost[:, 2 * QBS:D]
                    )

            for qb in range(NQB):
                stt = attn[qb]
                qsl = slice(qb * QBS, (qb + 1) * QBS)
                n = len(stt["kt_order"])
                # qb3's kt loop absorbs the first three out_proj(2) st-blocks
                # (spaced 3 pairs apart — ACT is exp-loaded in this window);
                # st3 stays for the tail, where it covers the recip/mul
                # latency before out_proj(3).
                op_pts = {8: 0, 11: 1, 14: 2} if qb == 3 else {}
                while stt["emitted"] < n:
                    emit_score(stt)
                    # The first PV of a block allocates the shared ot psum
                    # slot, which waits on the PREVIOUS block's O^T scale —
                    # hold it back until enough score pairs are queued to
                    # cover that latency.
                    min_emit = 5 if (qb > 0 and n > 5) else OT_LAG + 1
                    while stt["pv_done"] <= stt["emitted"] - 1 - OT_LAG and (
                        stt["pv_done"] > 0 or stt["emitted"] >= min_emit
                    ):
                        emit_pv(stt)
                    if stt["emitted"] in op_pts:
                        out_proj_st(2, [op_pts[stt["emitted"]]])
                while stt["pv_done"] < n:
                    emit_pv(stt)

                # softmax denominator: merge lanes (DVE) first, then pre-roll
                # the NEXT block's first score pairs so the PE has work while
                # the merge completes; the ones-matmul per head lands the
                # denominator broadcast across all partitions, then a fast
                # approximate reciprocal and the O^T scale.
                lanes = stt["lanes"]
                if lanes[1] is not None:
                    nc.vector.tensor_add(lanes[0], lanes[0], lanes[1])
                if qb + 1 < NQB:
                    for _ in range(PREROLL):
                        emit_score(attn[qb + 1])
                den_tiles = []
                for h in range(NHC):
                    dps = ps_c.tile([P, QBS], F32, tag="c", name="den_ps")
                    nc.tensor.matmul(
                        dps, lhsT=ones_sb, rhs=lanes[0][:, h, :],
                        start=True, stop=True, skip_group_check=True,
                    )
                    den_tiles.append(dps)
                # recip+mul IMMEDIATELY after den on the DVE FIFO — before
                # any out_proj copies queue up — so the next block's first
                # PV (which recycles the ot psum slot) and out_proj(3)
                # (which needs otb) never wait on a buried mul.
                rcp = work.tile([P, NHC, QBS], F32, tag="rcp")
                for h in range(NHC):
                    nc.vector.reciprocal_approx_fast(
                        out=rcp[:, h, :], in_=den_tiles[h]
                    )
                nc.vector.tensor_mul(otb_sb[:, 0:NHC, qsl], stt["ot_ps"], rcp)
                if qb == 3:
                    # held-back out_proj(2) st3: PE work covering the
                    # recip+mul latency before out_proj(3) can start.
                    out_proj_st(2, [3], tail=True)
                # out_proj(qb-1) interleaved with the next block's score
                # pairs: independent matmuls between the op st-groups absorb
                # the psum-slot copy latency.
                if qb in (1, 2):
                    for s4 in range(4):
                        out_proj_st(qb - 1, [s4])
                        if attn[qb + 1]["emitted"] < len(
                            attn[qb + 1]["kt_order"]
                        ):
                            emit_score(attn[qb + 1])

            out_proj_st(NQB - 1, range(4), tail=True)
    # Populate .instr bytes for extended-inst InstISA subclasses (the
    # custom-DVE reciprocal) — raw Bass skips this pass and the NEFF
    # compiler then fails with "ISA wrong length".
    from concourse.library_overlay import lower_extended_insts

    lower_extended_insts(nc)
    if split_waits:
        _split_excess_waits(nc)
    return nc


_NC_CACHE = {}


def _get_nc():
    if "nc" not in _NC_CACHE:
        _NC_CACHE["nc"] = build_nc()
    return _NC_CACHE["nc"]


def _rotation_matrix_T():
    # rot(x)[2i] = -x[2i+1]; rot(x)[2i+1] = x[2i].  R[i,j] coefficient of x[j].
    R = np.zeros((HD, HD), np.float32)
    idx = np.arange(0, HD, 2)
    R[idx, idx + 1] = -1.0
    R[idx + 1, idx] = 1.0
    return np.ascontiguousarray(R.T)


def prepare_in_maps(hidden_states, sin, cos, Wq, Wk, Wv, Wo):
    hidden_states = np.asarray(hidden_states, dtype=np.float32)
    sin = np.asarray(sin, dtype=np.float32)
    cos = np.asarray(cos, dtype=np.float32)
    Wq = np.asarray(Wq, dtype=np.float32)
    Wk = np.asarray(Wk, dtype=np.float32)
    Wv = np.asarray(Wv, dtype=np.float32)
    Wo = np.asarray(Wo, dtype=np.float32)

    # hT pre-arranged as [p, qb, kd, s_in_block]: partition line fully
    # contiguous per (qb, kd) chunk.
    hT = hidden_states[0].T  # [D, S]
    hT_r = np.ascontiguousarray(
        hT.reshape(KD, P, NQB, QBS).transpose(1, 2, 0, 3)
    ).astype(NPBF16)
    ct = np.ascontiguousarray(np.repeat(cos, 2, axis=1).T).astype(NPBF16)
    stm = np.ascontiguousarray(np.repeat(sin, 2, axis=1).T).astype(NPBF16)
    rot = _rotation_matrix_T().astype(NPBF16)

    def warr(w_slice):  # [D, DC] -> [p, kd, e]
        return np.ascontiguousarray(
            w_slice.reshape(KD, P, DC).transpose(1, 0, 2)
        ).astype(NPBF16)

    in_maps = []
    for c in range(N_CORES):
        e0 = c * DC
        wo_s = Wo[:, e0:e0 + DC].T  # [DC, D]
        in_maps.append(
            {
                "hT": hT_r,
                "wq": warr(Wq[e0:e0 + DC, :].T),
                "wk": warr(Wk[e0:e0 + DC, :].T),
                "wv": warr(Wv[e0:e0 + DC, :].T),
                "wo": np.ascontiguousarray(
                    wo_s.reshape(NHC, P, D).transpose(1, 0, 2)
                ).astype(NPBF16),
                "ct": ct,
                "st": stm,
                "rot": rot,
            }
        )
    return in_maps


def kernel(hidden_states, attention_mask, sin, cos, Wq, Wk, Wv, Wo):
    in_maps = prepare_in_maps(hidden_states, sin, cos, Wq, Wk, Wv, Wo)
    nc = _get_nc()
    res = run_bass_kernel_spmd(nc, in_maps, list(range(N_CORES)))
    out = res.results[0]["out"].astype(np.float32)
    for c in range(1, N_CORES):
        out += res.results[c]["out"].astype(np.float32)
    return out[None]
